# revision 4
# baseline (speedup 1.0000x reference)
"""Grouped-Query Attention (B=2, T=2048, C=2048, 16 Q heads / 4 KV heads,
D=128) on 8 Trainium2 NeuronCores.

Sharding: core (b, g) for b in {0,1}, g in {0..3} handles batch b and KV head
g (= query heads 4g..4g+3). Each core computes its 4 heads' attention plus the
partial output projection against its 512-row slice of Wo; the host sums the
4 partials per batch (the "all-reduce" of the o_proj, done in numpy).

All matmul operands are bf16 (host-cast); PSUM accumulation stays fp32.

Key scheduling ideas (v2, from trace analysis of the two-phase baseline):
- The softmax denominator is NOT computed with ones-lhsT matmuls per s-tile
  (that cost ~70k PE cycles); instead DVE accumulates the exp'd score tiles
  lane-wise into a per-head SBUF accumulator (bf16 adds) and a single
  [128,512] ones-matmul per head does the final cross-partition reduce.
  Saves ~26us of PE time for ~45us of otherwise-idle DVE time.
- Single merged pipeline: the projection passes for t-block tb+1 are
  interleaved between the attention head-chunks of t-block tb. This spreads
  the ScalarE exp stream (~86us total, 1 elem/cycle/lane) across the whole
  kernel instead of bunching it into an attention-only phase where it would
  gate the PE after the den offload.
- PSUM bank roles: bk0/bk1 = per-head oT (alternating), bk2/bk3 = shared
  rotation for projection passes / v-transposes / o_proj accumulators,
  bk4 = den final reduce, bk5/6/7 = score-tile rotation (LA=3 lookahead).
- 8 warmup matmuls on a zeroed tile run during the initial DMA wait so the
  PE HAM clock-gate is already at 2.4 GHz when the first real matmul issues.
- First two x/weight chunks are DMA'd in 4 partition-range pieces each so
  they spread over many DMA queues and the first projection matmul unblocks
  ~2-3us earlier.
- o_proj output rows for the last t-block are DMA'd per-512-column chunk as
  soon as each PSUM->SBUF copy lands, shrinking the end-of-kernel drain.
- Diagonal (causal-boundary) s-tiles get an additive -512*(s>t) triangular
  matmul folded into the score accumulation on the PE (exp then underflows
  to ~0, so the masked region contributes nothing to den/PV).
"""
import sys

sys.path.insert(0, "/opt/trn_rl_repo")

import numpy as np
import ml_dtypes

B, T, C = 2, 2048, 2048
NUM_HEADS, NUM_KV_HEADS, HEAD_DIM = 16, 4, 128
G = NUM_HEADS // NUM_KV_HEADS  # 4 query heads per core
SCALE = float(HEAD_DIM) ** -0.5
TB = 512  # t-block (matmul moving free dim)
NTB = T // TB  # 4
ST = 128  # s-tile
NST = T // ST  # 16
NCT = C // 128  # 16 contraction tiles
LA = 3  # score-matmul lookahead (s-tiles in flight past exp)
DEFER = 2  # head end-chain deferral (in acc pops)

SWAP_MASK = [i ^ 1 for i in range(32)]
BF = ml_dtypes.bfloat16

_nc_cache: dict = {}

# plan entry kinds
FULL, DIAG, GEN = 0, 1, 2


def _classify_mask(mask2d: np.ndarray):
    """mask2d[t, s] bool. Returns (plan, mask_tiles).

    plan[tb] = tuple of (s_tile_idx, w0, kind, mask_id). w0 is the t-window
    start within the t-block (columns < w0 are entirely masked for this
    s-tile). kind: FULL (no mask work), DIAG (additive triangular mask on
    the first 128 window columns), GEN (per-tile 0/1 multiply over the
    whole window; mask_id indexes mask_tiles)."""
    tri = (np.arange(ST)[:, None] <= np.arange(ST)[None, :])
    plan = []
    uniq: dict = {}
    tiles = []
    for tb in range(NTB):
        sub_t = mask2d[tb * TB : (tb + 1) * TB]  # [TB(t), T(s)]
        entries = []
        for s in range(NST):
            sub = sub_t[:, s * ST : (s + 1) * ST]  # [TB(t), ST(s)]
            if sub.all():
                entries.append((s, 0, FULL, None))
                continue
            if not sub.any():
                continue
            m = sub.T  # [s, t]
            w0 = 0
            while w0 + ST <= TB and not m[:, w0 : w0 + ST].any():
                w0 += ST
            win = m[:, w0:]
            if (
                win.shape[1] >= ST
                and (win[:, :ST] == tri).all()
                and win[:, ST:].all()
            ):
                entries.append((s, w0, DIAG, None))
                continue
            tile_m = np.zeros((ST, TB), dtype=np.float32)
            tile_m[:, : TB - w0] = win.astype(np.float32)
            key = (w0, tile_m.tobytes())
            mid = uniq.get(key)
            if mid is None:
                mid = len(tiles)
                uniq[key] = mid
                tiles.append(tile_m)
            entries.append((s, w0, GEN, mid))
        plan.append(tuple(entries))
    mask_tiles = (
        np.stack(tiles) if tiles else np.zeros((0, ST, TB), dtype=np.float32)
    )
    return tuple(plan), mask_tiles


def _build(plan, n_masks):
    import concourse.bacc as bacc
    import concourse.mybir as mybir
    import concourse.tile as tile

    F32 = mybir.dt.float32
    BF16 = mybir.dt.bfloat16
    Exp = mybir.ActivationFunctionType.Exp

    nc = bacc.Bacc()

    xT_d = nc.declare_dram_parameter("xT", [C, T], BF16, isOutput=False)
    wqkv_d = nc.declare_dram_parameter(
        "wqkv", [C, (G + 2) * HEAD_DIM], BF16, isOutput=False
    )
    wo_d = nc.declare_dram_parameter("wo", [G * HEAD_DIM, C], BF16, isOutput=False)
    on_d = nc.declare_dram_parameter("ones", [128, 128], BF16, isOutput=False)
    id_d = nc.declare_dram_parameter("ident", [128, 128], BF16, isOutput=False)
    tr_d = nc.declare_dram_parameter("tri", [ST, ST], BF16, isOutput=False)
    ct_d = nc.declare_dram_parameter("ctab", [HEAD_DIM, T], BF16, isOutput=False)
    st_d = nc.declare_dram_parameter("stab", [HEAD_DIM, T], BF16, isOutput=False)
    if n_masks:
        mk_d = nc.declare_dram_parameter(
            "masks", [n_masks * ST, TB], BF16, isOutput=False
        )
    out_d = nc.declare_dram_parameter("out", [T, C], BF16, isOutput=True)

    with tile.TileContext(nc) as tc:
        const = tc.alloc_tile_pool(name="const", bufs=1)
        wop = tc.alloc_tile_pool(name="wop", bufs=1)
        qkv = tc.alloc_tile_pool(name="qkv", bufs=1)
        xp = tc.alloc_tile_pool(name="xp", bufs=1)

        # --- startup DMAs. First two contraction chunks are split into 4
        # partition-range pieces so they land across many queues and the
        # first projection matmul unblocks early. ---
        wqkv_sb = [
            wop.tile([128, (G + 2) * HEAD_DIM], BF16, name=f"wqkv{i}")
            for i in range(NCT)
        ]
        xts = [
            [xp.tile([128, TB], BF16, name=f"xt{tb}_{i}") for i in range(NCT)]
            for tb in range(NTB)
        ]
        for i in range(NCT):
            sl = slice(i * 128, (i + 1) * 128)
            if i < 2:
                for r in range(4):
                    rs = slice(r * 32, (r + 1) * 32)
                    rg = slice(i * 128 + r * 32, i * 128 + (r + 1) * 32)
                    nc.sync.dma_start(out=wqkv_sb[i][rs, :], in_=wqkv_d.ap()[rg, :])
                    nc.sync.dma_start(out=xts[0][i][rs, :], in_=xT_d.ap()[rg, :TB])
            else:
                nc.sync.dma_start(out=wqkv_sb[i], in_=wqkv_d.ap()[sl, :])
                nc.sync.dma_start(out=xts[0][i], in_=xT_d.ap()[sl, :TB])

        ctab = const.tile([HEAD_DIM, T], BF16, name="ctab")
        stab = const.tile([HEAD_DIM, T], BF16, name="stab")
        nc.sync.dma_start(out=ctab, in_=ct_d.ap())
        nc.sync.dma_start(out=stab, in_=st_d.ap())
        ones_sb = const.tile([128, 128], BF16, name="ones_sb")
        ident = const.tile([128, 128], BF16, name="ident")
        trineg = const.tile([ST, ST], BF16, name="trineg")
        nc.sync.dma_start(out=ones_sb, in_=on_d.ap())
        nc.sync.dma_start(out=ident, in_=id_d.ap())
        nc.sync.dma_start(out=trineg, in_=tr_d.ap())
        if n_masks:
            msk_sb = const.tile([ST, n_masks * TB], BF16, name="msk_sb")
            for i in range(n_masks):
                nc.sync.dma_start(
                    out=msk_sb[:, i * TB : (i + 1) * TB],
                    in_=mk_d.ap()[i * ST : (i + 1) * ST, :],
                )
        for tb in range(1, NTB):
            for i in range(NCT):
                sl = slice(i * 128, (i + 1) * 128)
                nc.sync.dma_start(
                    out=xts[tb][i], in_=xT_d.ap()[sl, tb * TB : (tb + 1) * TB]
                )
        wo_sb = [wop.tile([128, C], BF16, name=f"wo{h}") for h in range(G)]
        for h in range(G):
            nc.sync.dma_start(out=wo_sb[h], in_=wo_d.ap()[h * 128 : (h + 1) * 128, :])

        qT = [
            [qkv.tile([128, TB], BF16, name=f"qT{h}_{tb}") for tb in range(NTB)]
            for h in range(G)
        ]
        kT = [qkv.tile([128, TB], BF16, name=f"kT{tb}") for tb in range(NTB)]
        vT = [qkv.tile([128, TB], BF16, name=f"vT{tb}") for tb in range(NTB)]
        vch = [qkv.tile([128, 128], BF16, name=f"v{s}") for s in range(NST)]

        ps = tc.alloc_tile_pool(name="ps", bufs=1, space="PSUM")

        def bank(tag):
            return ps.tile([128, TB], F32, name=tag, tag=tag)

        rpool = tc.alloc_tile_pool(name="rpool", bufs=3)

        # --- HAM warmup: ~3.4us of dummy matmuls during the input-DMA wait
        # so the PE clock-gate is at 8/8 when real work starts ---
        wz = const.tile([128, TB], BF16, name="wz")
        nc.vector.memset(wz, 0.0)
        for i in range(8):
            nc.tensor.matmul(
                bank(f"bk{6 + (i % 2)}"),
                lhsT=wz[:, :128],
                rhs=wz,
                start=True,
                stop=True,
                skip_group_check=True,
            )

        # ---- stage-1 emitters ----
        state = {"score": 0, "head": 0, "pb": 0, "ncopy": 0}

        def pbank():
            state["pb"] += 1
            return bank(f"bk{2 + (state['pb'] % 2)}")

        def emit_v_post(tb, v_ps):
            nc.vector.tensor_copy(vT[tb], v_ps)
            for r in range(4):
                s = 4 * tb + r
                vtp = pbank().bitcast(BF16)[:, :128]
                nc.tensor.transpose(vtp, vT[tb][:, r * 128 : (r + 1) * 128], ident)
                nc.vector.tensor_copy(vch[s], vtp)

        def emit_rope(tb, dst, src_ps):
            tsl = slice(tb * TB, (tb + 1) * TB)
            nc.vector.tensor_copy(dst, src_ps)
            swp = rpool.tile([128, TB], BF16, name="swp", tag="swp")
            tmp = rpool.tile([128, TB], BF16, name="tmp", tag="tmp")
            nc.vector.stream_shuffle(swp, dst, SWAP_MASK)
            nc.vector.tensor_mul(tmp, dst, ctab[:, tsl])
            nc.vector.tensor_mul(swp, swp, stab[:, tsl])
            nc.vector.tensor_add(dst, tmp, swp)

        def emit_proj0():
            # ci-major: tb0 is paced by the input DMA, so touch each
            # freshly-arrived chunk with all 6 matmuls at once
            q_ps = [bank(f"bk{h}") for h in range(G)]
            k_ps = bank("bk4")
            v_ps = bank("bk5")
            for ci in range(NCT):
                first, last = ci == 0, ci == NCT - 1
                rhs = xts[0][ci]
                w = wqkv_sb[ci]
                nc.tensor.matmul(
                    v_ps, lhsT=w[:, 640:768], rhs=rhs, start=first, stop=last
                )
                nc.tensor.matmul(
                    k_ps, lhsT=w[:, 512:640], rhs=rhs, start=first, stop=last
                )
                for h in range(G):
                    nc.tensor.matmul(
                        q_ps[h],
                        lhsT=w[:, h * 128 : (h + 1) * 128],
                        rhs=rhs,
                        start=first,
                        stop=last,
                    )
            emit_v_post(0, v_ps)
            emit_rope(0, kT[0], k_ps)
            for h in range(G):
                emit_rope(0, qT[h][0], q_ps[h])

        def emit_pass(tbp, p):
            # one projection pass (v / k / q_h) for t-block tbp, into a
            # rotating bank; the DVE consumer follows immediately
            dst = pbank()
            if p == 0:
                wsl = slice(640, 768)
            elif p == 1:
                wsl = slice(512, 640)
            else:
                h = p - 2
                wsl = slice(h * 128, (h + 1) * 128)
            for ci in range(NCT):
                nc.tensor.matmul(
                    dst,
                    lhsT=wqkv_sb[ci][:, wsl],
                    rhs=xts[tbp][ci],
                    start=ci == 0,
                    stop=ci == NCT - 1,
                )
            if p == 0:
                emit_v_post(tbp, dst)
            elif p == 1:
                emit_rope(tbp, kT[tbp], dst)
            else:
                emit_rope(tbp, qT[p - 2][tbp], dst)

        # ---- attention + o_proj emitters ----
        p2sb = tc.alloc_tile_pool(name="p2sb", bufs=6)
        accp = tc.alloc_tile_pool(name="accp", bufs=3)
        phd = tc.alloc_tile_pool(name="phd", bufs=2)
        p3sb = tc.alloc_tile_pool(name="p3sb", bufs=3)
        outp = tc.alloc_tile_pool(name="outp", bufs=10)
        oT_live: dict = {}
        ctx: dict = {}

        def emit_oproj(tb):
            oT_sbs = oT_live.pop(tb)
            fine = tb == NTB - 1  # per-cb DMA to shrink the final drain
            for tch in range(TB // 128):
                t0 = tb * TB + tch * 128
                osb = (
                    None
                    if fine
                    else p3sb.tile([128, C], BF16, name="osb", tag="osb")
                )
                for cb in range(C // 512):
                    ops = pbank()
                    for h in range(G):
                        nc.tensor.matmul(
                            ops,
                            lhsT=oT_sbs[h][:, tch * 128 : (tch + 1) * 128],
                            rhs=wo_sb[h][:, cb * 512 : (cb + 1) * 512],
                            start=h == 0,
                            stop=h == G - 1,
                        )
                    if fine:
                        # separate per-cb tiles: each DMA only depends on its
                        # own copy, so copies and DMAs pipeline
                        dst = p3sb.tile(
                            [128, 512], BF16, name="osbf", tag=f"osbf{cb % 2}"
                        )
                    else:
                        dst = osb[:, cb * 512 : (cb + 1) * 512]
                    # alternate PSUM->SBUF copies between ScalarE and DVE
                    if state["ncopy"] % 2 == 0:
                        nc.scalar.copy(dst, ops)
                    else:
                        nc.vector.tensor_copy(dst, ops)
                    state["ncopy"] += 1
                    if fine:
                        for q in range(4):
                            rs = slice(q * 32, (q + 1) * 32)
                            nc.sync.dma_start(
                                out=out_d.ap()[
                                    t0 + q * 32 : t0 + (q + 1) * 32,
                                    cb * 512 : (cb + 1) * 512,
                                ],
                                in_=dst[rs, :],
                            )
                if not fine:
                    for q in range(4):
                        nc.sync.dma_start(
                            out=out_d.ap()[t0 + q * 32 : t0 + (q + 1) * 32, :],
                            in_=osb[q * 32 : (q + 1) * 32, :],
                        )

        def emit_score(it):
            tb, h, idx = it
            entries = plan[tb]
            s, w0, kind, mid = entries[idx]
            if idx == 0:
                ctx[(tb, h)] = {
                    "oT": bank(f"bk{0 + (state['head'] % 2)}"),
                    "acc": accp.tile(
                        [ST, TB], BF16, name="acc", tag=f"acc{state['head'] % 3}"
                    ),
                    "eps": {},
                }
                state["head"] += 1
            stp = bank(f"bk{5 + (state['score'] % 3)}")
            state["score"] += 1
            diag = kind == DIAG
            nc.tensor.matmul(
                stp[:, w0:],
                lhsT=kT[s // 4][:, (s % 4) * 128 : (s % 4 + 1) * 128],
                rhs=qT[h][tb][:, w0:],
                start=True,
                stop=not diag,
                skip_group_check=diag,
            )
            if diag:
                # additive -512*(s>t) triangular mask folded into the score
                # accumulation on the PE; exp then underflows to ~e-18
                nc.tensor.matmul(
                    stp[:, w0 : w0 + ST],
                    lhsT=ident,
                    rhs=trineg,
                    start=False,
                    stop=True,
                    skip_group_check=True,
                )
            ep = p2sb.tile([ST, TB], BF16, name="ep", tag="ep")
            nc.scalar.activation(ep[:, w0:], stp[:, w0:], Exp, scale=SCALE)
            if kind == GEN:
                nc.vector.tensor_mul(
                    ep[:, w0:],
                    ep[:, w0:],
                    msk_sb[:, mid * TB : mid * TB + TB - w0],
                )
            ctx[(tb, h)]["eps"][idx] = ep

        chainq: list = []  # [countdown, closure]

        def emit_acc(it):
            tb, h, idx = it
            entries = plan[tb]
            s, w0, kind, mid = entries[idx]
            c = ctx[(tb, h)]
            ep = c["eps"].pop(idx)
            first, last = idx == 0, idx == len(entries) - 1
            nc.tensor.matmul(
                c["oT"][:, w0:],
                lhsT=vch[s],
                rhs=ep[:, w0:],
                start=first,
                stop=last,
                skip_group_check=True,
            )
            # DVE lane-wise denominator accumulation (replaces the per-tile
            # ones-matmul on the PE)
            if first:
                nc.vector.tensor_copy(c["acc"], ep)
            else:
                nc.vector.tensor_add(
                    c["acc"][:, w0:], c["acc"][:, w0:], ep[:, w0:]
                )
            if last:
                fast = tb == NTB - 1 and h == G - 1

                def chain(tb=tb, h=h, c=c, fast=fast):
                    den_ps = bank("bk4")
                    nc.tensor.matmul(
                        den_ps, lhsT=ones_sb, rhs=c["acc"], start=True, stop=True
                    )
                    oT_sb = outp.tile([128, TB], BF16, name="oT", tag="oT")
                    rcp = phd.tile([128, TB], F32, name="rcp", tag="rcp")
                    if fast:
                        # end of kernel: shortest chain, read PSUM directly
                        nc.vector.reciprocal_approx_fast(rcp, den_ps)
                        nc.vector.tensor_mul(oT_sb, c["oT"], rcp)
                    else:
                        den_sb = phd.tile([128, TB], F32, name="den_sb", tag="den_sb")
                        nc.scalar.copy(den_sb, den_ps)
                        oT_f = phd.tile([128, TB], F32, name="oT_f", tag="oT_f")
                        nc.scalar.copy(oT_f, c["oT"])
                        nc.vector.reciprocal_approx_fast(rcp, den_sb)
                        nc.vector.tensor_mul(oT_sb, oT_f, rcp)
                    oT_live.setdefault(tb, {})[h] = oT_sb
                    del ctx[(tb, h)]

                if fast:
                    chain()
                else:
                    chainq.append([DEFER, chain])

        from collections import deque

        pend = deque()

        def tick_chains():
            while chainq and chainq[0][0] <= 0:
                chainq.pop(0)[1]()
            for e in chainq:
                e[0] -= 1

        def emit_entry(it):
            emit_score(it)
            pend.append(it)
            if len(pend) > LA:
                emit_acc(pend.popleft())
                tick_chains()

        def flush():
            while pend:
                emit_acc(pend.popleft())
                tick_chains()
            while chainq:
                chainq.pop(0)[1]()

        # ---- master emission ----
        emit_proj0()
        for tb in range(NTB):
            heads = [
                [("entry", tb, h, idx) for idx in range(len(plan[tb]))]
                for h in range(G)
            ]
            opj = [("oproj", tb - 1)] if tb >= 1 else []
            if tb + 1 < NTB:
                seq = (
                    [("pass", tb + 1, 0), ("pass", tb + 1, 1)]
                    + heads[0]
                    + [("pass", tb + 1, 2)]
                    + heads[1]
                    + opj
                    + [("pass", tb + 1, 3)]
                    + heads[2]
                    + [("pass", tb + 1, 4)]
                    + heads[3]
                    + [("pass", tb + 1, 5)]
                )
            else:
                seq = heads[0] + heads[1] + opj + heads[2] + heads[3]
            for atom in seq:
                if atom[0] == "entry":
                    emit_entry((atom[1], atom[2], atom[3]))
                elif atom[0] == "pass":
                    emit_pass(atom[1], atom[2])
                else:
                    emit_oproj(atom[1])
        flush()
        emit_oproj(NTB - 1)

        outp.release()
        p3sb.release()
        phd.release()
        accp.release()
        p2sb.release()
        rpool.release()
        ps.release()
        xp.release()
        qkv.release()
        wop.release()
        const.release()

    nc.compile()
    return nc


def _prep_inputs(x, cos, sin, Wq, Wk, Wv, Wo, mask_tiles, n_masks):
    cos = np.asarray(cos, dtype=np.float32).reshape(T, HEAD_DIM // 2)
    sin = np.asarray(sin, dtype=np.float32).reshape(T, HEAD_DIM // 2)
    ctab = np.ascontiguousarray(np.repeat(cos, 2, axis=1).T).astype(BF)  # [128, T]
    s2 = np.repeat(sin, 2, axis=1)
    s2[:, 0::2] *= -1.0
    stab = np.ascontiguousarray(s2.T).astype(BF)
    trineg = (-512.0 * (np.arange(ST)[:, None] > np.arange(ST)[None, :])).astype(BF)

    xTb = [
        np.ascontiguousarray(np.asarray(x[b], dtype=np.float32).T).astype(BF)
        for b in range(B)
    ]
    in_maps = []
    for core in range(8):
        b, g = divmod(core, NUM_KV_HEADS)
        wqkv = np.concatenate(
            [
                Wq[:, g * 512 : (g + 1) * 512],
                Wk[:, g * 128 : (g + 1) * 128],
                Wv[:, g * 128 : (g + 1) * 128],
            ],
            axis=1,
        )
        m = {
            "xT": xTb[b],
            "wqkv": np.ascontiguousarray(wqkv).astype(BF),
            "wo": np.ascontiguousarray(Wo[g * 512 : (g + 1) * 512, :]).astype(BF),
            "ctab": ctab,
            "stab": stab,
            "ones": np.ones((128, 128), dtype=BF),
            "ident": np.eye(128, dtype=BF),
            "tri": trineg,
        }
        if n_masks:
            m["masks"] = mask_tiles.reshape(n_masks * ST, TB).astype(BF)
        in_maps.append(m)
    return in_maps


def kernel(x, cos, sin, mask, Wq, Wk, Wv, Wo, _trace=False, _result_box=None):
    from concourse.bass_utils import run_bass_kernel_spmd

    mask2d = np.asarray(mask).reshape(T, T).astype(bool)
    plan, mask_tiles = _classify_mask(mask2d)
    n_masks = int(mask_tiles.shape[0])

    key = (plan, n_masks)
    nc = _nc_cache.get(key)
    if nc is None:
        nc = _build(plan, n_masks)
        _nc_cache[key] = nc

    in_maps = _prep_inputs(x, cos, sin, Wq, Wk, Wv, Wo, mask_tiles, n_masks)
    res = run_bass_kernel_spmd(nc, in_maps, core_ids=list(range(8)), trace=_trace)
    if _result_box is not None:
        _result_box.append(res)

    out = np.zeros((B, T, C), dtype=np.float32)
    for core in range(8):
        b = core // NUM_KV_HEADS
        out[b] += res.results[core]["out"].astype(np.float32)
    return out
